# revision 1
# baseline (speedup 1.0000x reference)
"""Multi-head attention (B=4, S=2048, D=768, H=12, d=64) on 8 trn2 NeuronCores.

Sharding: core c handles batch b = c//2 and head-group g = c%2 (6 heads each).
Per core: column-parallel QKV projections (wq/wk/wv column slices), full
attention for its 6 heads, row-parallel output projection (wo row slice).
The two partial outputs per batch are reduced on the host (+ bo and the
bv @ wo correction, exact because softmax rows sum to 1).

The kernel is ACT-bound: exp of 25.2M scores costs ~199us on the scalar
engine and nothing else can compute exp. Everything is arranged to hide
under that: scores run as fp8e4m3 DoubleRow matmuls (half PE cost, the
e4m3 q/k rounding costs ~1.36e-2 rel err), projections stream in bf16
and are interleaved quantum-by-quantum into the attention sk-loop via a
static slot schedule so the PE never bursts long enough to starve the
ACT engine. e/V tiles are bf16 (DVE 2x adds for the softmax partials,
split into short chains), weights+constants arrive as two packed DMAs
ordered for the serial DMA device, and a junk-matmul spree warms the PE
p-state during the ramp. ctx / denominator-reduce / output projection
stay float32r. TimelineSim: 270.9us (baseline 308.6us).
"""
import sys

for _p in ("/opt/trn_rl_repo", "/root/.axon_site/_ro/trn_rl_repo"):
    if _p not in sys.path:
        sys.path.append(_p)

import numpy as np
import ml_dtypes

import concourse.bass as bass  # noqa: F401  (engine namespaces live on the nc object)
import concourse.bacc as bacc
import concourse.mybir as mybir
import concourse.tile as tile
from concourse.bass_utils import run_bass_kernel_spmd

B, S, D = 4, 2048, 768
NUM_HEADS, HEAD = 12, 64
NCORES = 8
HPC = NUM_HEADS // 2          # 6 heads per core
MC = HPC * HEAD               # 384 per-core projection cols
KT = D // 128                 # 6 contraction k-tiles
MT = MC // 128                # 3 head-pair tiles
ST = S // 128                 # 16 key tiles
SQW = 512                     # psum bank width in f32
CW = 1024                     # attention q-chunk width
NCH = S // CW                 # 2 q-chunks
CHW = 512                     # x streaming chunk width
NXC = S // CHW                # 4 x chunks
NACC = 2                      # denominator partial chains per head

F32 = mybir.dt.float32
F32R = mybir.dt.float32r
BF16 = mybir.dt.bfloat16
F8 = mybir.dt.float8e4
DR = mybir.MatmulPerfMode.DoubleRow
EXP = mybir.ActivationFunctionType.Exp
ADD = mybir.AluOpType.add
MULT = mybir.AluOpType.mult

_NC = None
LAST_RESULTS = None
_LAST_IN_MAPS = None


def _build(loop=None):
    nc = bacc.Bacc("TRN2", target_bir_lowering=False, debug=False,
                   num_devices=NCORES)
    xqt = nc.declare_dram_parameter("xqt", [D, S], BF16, isOutput=False)
    xkt = nc.declare_dram_parameter("xkt", [D, S], BF16, isOutput=False)
    xvt = nc.declare_dram_parameter("xvt", [D, S], BF16, isOutput=False)
    # bf16 pack: [wq_m0 | bq | bk | cstb(68) | wq_m1,m2 | wk | wv] so one
    # small leading DMA unblocks the first projection; f32 pack: [wo | cst]
    wpk16 = nc.declare_dram_parameter("wpk16", [128, 3 * KT * MC + 74], BF16,
                                      isOutput=False)
    wpk32 = nc.declare_dram_parameter("wpk32", [128, MT * D + 354],
                                      F32R, isOutput=False)
    out = nc.declare_dram_parameter("out", [S, D], F32, isOutput=True)

    with tile.TileContext(nc) as tc:
        if loop:
            with tc.For_i(0, loop, 1):
                _emit(nc, tc, xqt, xkt, xvt, wpk16, wpk32, out)
        else:
            _emit(nc, tc, xqt, xkt, xvt, wpk16, wpk32, out)
    nc.compile()
    return nc


def _emit(nc, tc, xqt, xkt, xvt, wpk16, wpk32, out):
    ctx_lp = nc.allow_low_precision(reason="fp8/bf16 tiles feed the PE; accumulation stays fp32 in PSUM")
    ctx_lp.__enter__()
    with (
        tc.tile_pool(name="cst", bufs=1) as cst_pool,
        tc.tile_pool(name="q8p", bufs=1) as q8_pool,
        tc.tile_pool(name="vp", bufs=ST) as v_pool,
        tc.tile_pool(name="ctxp", bufs=MT) as ctx_pool,
        tc.tile_pool(name="accp", bufs=2) as acc_pool,
        tc.tile_pool(name="rp", bufs=2) as r_pool,
        tc.tile_pool(name="wop", bufs=1) as wo_pool,
        tc.tile_pool(name="outp", bufs=4) as out_pool,
        tc.tile_pool(name="wp", bufs=1) as w_pool,
        tc.tile_pool(name="xtp", bufs=10) as xt_pool,
        tc.tile_pool(name="stgp", bufs=2) as stg_pool,
        tc.tile_pool(name="psS", bufs=2, space="PSUM") as psS,
        tc.tile_pool(name="psC", bufs=3, space="PSUM") as psC,
        tc.tile_pool(name="psM", bufs=1, space="PSUM") as psM,
    ):
        psA = psM
        # packed weight/constant tiles; views are carved out below.
        # cst layout: [:,0:1]=ones  [:,1:34]=[32 zero cols|ones]
        # [:,34:98]=ones  [:,98:226]=[64 zero cols|64 one cols]
        pk16h = cst_pool.tile([128, KT * 128], BF16, tag="pk16h")
        pk16 = cst_pool.tile([128, KT * 128 + KT * MC + 74], BF16,
                             tag="pk16")
        pk16b = cst_pool.tile([128, 4 * KT * 128], BF16, tag="pk16b")
        pk32 = cst_pool.tile([128, MT * D + 354], F32R, tag="pk32")

        # fp8 DoubleRow q/k: [32*hp+p, head, i, s] = proj_col(64*head+32*i+p
        # of pair hp) at seq s; scores lhsT/rhs slice partitions 32*hp..+32.
        qt8 = q8_pool.tile([96, 2, 2, S], F8, tag="qt8")
        kt8 = q8_pool.tile([96, 2, 2, S], F8, tag="kt8")
        # V tiles: [key_in_block, hp, 2*HEAD] (even head | odd head)
        vt = [v_pool.tile([128, MT, 2 * HEAD], BF16, tag="v", name=f"vt{st}")
              for st in range(ST)]
        ctx = [ctx_pool.tile([128, S], F32R, tag="ctx", name=f"ctx{m}")
               for m in range(MT)]

        # pack A (pk16) = [wq-m0 | wk-m0 | wv(all) | bq | bk | cstb]:
        # everything the first head-pair chunk needs, in one leading DMA.
        # pack B (pk16b, separate tile - tile-granular dep tracking!) =
        # [wq-m1, wq-m2, wk-m1, wk-m2]. The DMA device is a serial
        # resource in the cost model, so emission order = priority.
        KM = KT * MC
        PRE = 2 * KT * 128 + KM + 74
        nc.sync.dma_start(out=pk16h, in_=wpk16[:, 0:KT * 128])
        nc.sync.dma_start(out=pk16, in_=wpk16[:, KT * 128:PRE])

        def w_ap(name, k, m):
            # per-partition layout within each weight block: [m, k, j]
            if m == 0:
                if name == "wq":
                    return pk16h[:, k * 128:(k + 1) * 128]
                return pk16[:, k * 128:(k + 1) * 128]
            o = (0 if name == "wq" else 2 * KT * 128) + \
                ((m - 1) * KT + k) * 128
            return pk16b[:, o:o + 128]

        bqk = cst_pool.tile([128, 2 * MT], F32, tag="bqk")
        bq_sb = bqk[:, 0:MT]
        bk_sb = bqk[:, MT:2 * MT]
        # cstb: [1:34] odd-head 33-col reduce (ones at last col -> row 32),
        # [34:67] even-head (ones at col 31 -> row 31)
        cstb = pk16[:, KT * 128 + KM + 6:KT * 128 + KM + 74]
        wv4 = pk16[:, KT * 128:KT * 128 + KM].rearrange(
            "p (m k j) -> p k m j", m=MT, k=KT)
        wo_sb = pk32[:, 0:MT * D].rearrange("p (t o) -> p t o", t=MT)
        cst = pk32[:, MT * D:MT * D + 354]

        # ---------------- streaming projections ----------------
        x_dram = {0: xqt, 1: xkt, 2: xvt}
        x_tiles = {}

        def load_x(inp, c):
            x = xt_pool.tile([128, KT, CHW], BF16, tag="xt",
                             name=f"x{inp}_{c}")
            nc.sync.dma_start(
                out=x,
                in_=x_dram[inp][:].rearrange("(t p) s -> p t s", p=128)[
                    :, :, c * CHW:(c + 1) * CHW])
            x_tiles[(inp, c)] = x

        pending_stg = []

        def emit_qk_mm(inp, m, c, pool=None):
            # quantum phase 1: the projection matmul chain
            x = x_tiles[(inp, c)]
            wname = "wq" if inp == 0 else "wk"
            pool = pool or psA
            ps = pool.tile([128, SQW] if pool is not psS else [128, CW],
                           F32, tag="psS" if pool is psS else "psM",
                           name=f"psp{inp}{m}{c}")
            ps = ps[:, 0:SQW]
            for k in range(KT):
                nc.tensor.matmul(ps, w_ap(wname, k, m),
                                 x[:, k, :], start=(k == 0), stop=(k == KT - 1))
            pending_stg.append((inp, m, c, ps))

        def flush_stg():
            # quantum phase 2: fp8 quantize + shuffle-DMAs; runs one slot
            # later so the DVE op sits at the queue head, not behind adds
            while pending_stg:
                inp, m, c, ps = pending_stg.pop(0)
                bias_sb = bq_sb if inp == 0 else bk_sb
                dst = qt8 if inp == 0 else kt8
                stg = stg_pool.tile([128, CHW], F8, tag="stg")
                nc.vector.tensor_scalar_add(stg, ps, bias_sb[:, m:m + 1])
                for head in range(2):
                    for i in range(2):
                        nc.sync.dma_start(
                            out=dst[32 * m:32 * m + 32, head, i,
                                    c * CHW:(c + 1) * CHW],
                            in_=stg[64 * head + 32 * i:
                                    64 * head + 32 * i + 32, :])

        def emit_qk(inp, m, c):
            emit_qk_mm(inp, m, c)
            flush_stg()

        def emit_v(c, half):
            # two key-blocks of the V projection
            x = x_tiles[(2, c)]
            for st8 in (2 * half, 2 * half + 1):
                st = c * (CHW // 128) + st8
                ps = psA.tile([128, SQW], F32, tag="psM", name=f"psv{st}")
                for k in range(KT):
                    nc.tensor.matmul(ps[:, 0:MC],
                                     x[:, k, st8 * 128:(st8 + 1) * 128],
                                     wv4[:, k],
                                     start=(k == 0), stop=(k == KT - 1))
                psv = ps[:, 0:MC].rearrange("p (t two d) -> p t two d",
                                            two=2, d=HEAD)
                nc.vector.tensor_copy(vt[st][:, :, 0:HEAD], psv[:, :, 0])
                nc.vector.tensor_copy(vt[st][:, :, HEAD:], psv[:, :, 1])

        # Static slot schedule: work items emitted at the top of sk
        # iterations so projection quanta fill the PE slack under the
        # ACT-paced exp stream without ever bursting. Constraints held:
        # x-chunk 8-buf rotation distance, kt8/vt ready >=2 slots before
        # first use, qt8[hp] ready before chunk (sc,hp) starts.
        slot = {}

        def at(sc, hp, sk, *fns):
            slot.setdefault((sc, hp, sk), []).extend(fns)

        W = lambda inp, m, c: (lambda: emit_qk(inp, m, c))
        V = lambda c, h: (lambda: emit_v(c, h))
        L = lambda inp, c: (lambda: load_x(inp, c))
        # (0,0): finish q c0/c1 early (frees x slots for rotation), k m0 +
        # all vt in lockstep with the sk loop, k m1 prepped for (0,1).
        # Pending-norm chunks keep quanta off sk 3/5/7 (psM conflicts);
        # (0,0) has no pending norm so those slots are usable here.
        at(0, 0, 0, V(0, 0))
        at(0, 0, 1, V(0, 1))
        at(0, 0, 1, lambda: nc.sync.dma_start(out=pk16b, in_=wpk16[:, PRE:]))
        at(0, 0, 2, L(1, 2), W(1, 0, 1))
        at(0, 0, 3, lambda: nc.sync.dma_start(out=pk32, in_=wpk32[:]))
        at(0, 0, 3, L(2, 2), V(1, 0))
        at(0, 0, 4, W(0, 1, 0))
        at(0, 0, 5, L(1, 3), W(1, 0, 2))
        at(0, 0, 6, V(1, 1))
        at(0, 0, 7, V(2, 0))
        at(0, 0, 8, W(0, 2, 0))
        at(0, 0, 9, L(2, 3), W(1, 0, 3))
        at(0, 0, 10, V(2, 1))
        at(0, 0, 11, V(3, 0))
        at(0, 0, 12, W(1, 1, 0))
        at(0, 0, 13, W(0, 1, 1))
        at(0, 0, 14, V(3, 1))
        at(0, 0, 15, W(0, 2, 1))
        # (0,1): k m1 tail, k m2 for (0,2), q c2/c3 for sc1
        at(0, 1, 0, W(1, 1, 1))
        at(0, 1, 1, W(1, 1, 3))
        at(0, 1, 2, L(0, 2), W(1, 1, 2))
        at(0, 1, 4, L(0, 3))
        at(0, 1, 8, W(1, 2, 0))
        at(0, 1, 9, W(1, 2, 1))
        at(0, 1, 10, W(1, 2, 2))
        at(0, 1, 11, W(1, 2, 3))
        at(0, 1, 12, W(0, 0, 2))
        at(0, 1, 13, W(0, 0, 3))
        at(0, 1, 14, W(0, 1, 2))
        at(0, 1, 15, W(0, 1, 3))
        at(0, 2, 0, W(0, 2, 2))
        at(0, 2, 1, W(0, 2, 3))

        # ---------------- attention ----------------
        with (
            tc.tile_pool(name="ep0", bufs=8) as e0_pool,
            tc.tile_pool(name="ep1", bufs=8) as e1_pool,
        ):
            def emit_norm_reduce(state):
                # stage 1: partition-reduce matmuls + reciprocals
                q0, hp, ps_ch, acc0, acc1, nh4 = state
                rr = []
                for h4 in range(nh4):
                    qs = slice(h4 * SQW, (h4 + 1) * SQW)
                    ps_r = psM.tile([34, SQW], F32, tag="psM",
                                    name=f"psr{q0}{hp}{h4}")
                    for i in range(NACC):
                        nc.tensor.matmul(ps_r, cstb[:, 34:68],
                                         acc0[i][:, qs],
                                         start=(i == 0), stop=False,
                                         skip_group_check=True)
                        nc.tensor.matmul(ps_r[0:33, :], cstb[:, 1:34],
                                         acc1[i][:, qs],
                                         start=False, stop=(i == NACC - 1),
                                         skip_group_check=True)
                    r = r_pool.tile([2, SQW], F32R, tag="r")
                    nc.vector.reciprocal(r, ps_r[32:34, :])
                    rr.append(r)
                return rr

            def emit_norm_bcast(state, rr, h4):
                # stage 2: broadcast matmul + normalize into ctx (one half)
                q0, hp, ps_ch, acc0, acc1, nh4 = state
                r2 = rr[h4]
                ps_b = psM.tile([128, SQW], F32, tag="psM",
                                name=f"psb{q0}{hp}{h4}")
                nc.tensor.matmul(ps_b, cst[0:2, 226:354], r2,
                                 start=True, stop=True,
                                 skip_group_check=True)
                b_sb = r_pool.tile([128, SQW], F32, tag="bsb", bufs=2)
                nc.vector.tensor_copy(b_sb, ps_b)
                s0 = q0 + h4 * SQW
                nc.vector.tensor_tensor(ctx[hp][:, s0:s0 + SQW],
                                        ps_ch[h4], b_sb, op=MULT)

            def outproj_rounds(row0, nr, at_tail):
                # one round = one 128-row out block; two yields per round
                # mid-stream, interleaved into sk slots. At the tail, rounds
                # use the freed scores banks (psS) + psM and split their
                # psum->sbuf copies between ACT (idle then) and DVE.
                for st4 in range(nr):
                    s0 = row0 + st4 * 128
                    o_sb = out_pool.tile([128, D], F32, tag="osb")
                    if at_tail and st4 % 4 in (1, 2):
                        ps_o = psS.tile([128, CW], F32, tag="psS",
                                        name=f"psot{s0}")[:, 0:768]
                        for n0, nw in ((0, 512), (512, 256)):
                            for m in range(MT):
                                nc.tensor.matmul(
                                    ps_o[:, n0:n0 + nw],
                                    ctx[m][:, s0:s0 + 128],
                                    wo_sb[:, m, n0:n0 + nw],
                                    start=(m == 0), stop=(m == MT - 1))
                            yield
                        if st4 % 2:
                            nc.scalar.copy(o_sb, ps_o)
                        else:
                            nc.vector.tensor_copy(o_sb, ps_o)
                    else:
                        for n0, nw in ((0, 512), (512, 256)):
                            pool = psC if (at_tail and st4 % 4 == 3) else psM
                            ps_o = pool.tile([128, 512], F32,
                                            tag="psC" if pool is psC else "psM",
                                            name=f"pso{s0}{n0}")
                            for m in range(MT):
                                nc.tensor.matmul(
                                    ps_o[:, 0:nw],
                                    ctx[m][:, s0:s0 + 128],
                                    wo_sb[:, m, n0:n0 + nw],
                                    start=(m == 0), stop=(m == MT - 1))
                            if at_tail and st4 % 2:
                                nc.scalar.copy(o_sb[:, n0:n0 + nw],
                                               ps_o[:, 0:nw])
                            else:
                                nc.vector.tensor_copy(o_sb[:, n0:n0 + nw],
                                                      ps_o[:, 0:nw])
                            yield
                    eng = nc.sync if at_tail else nc.gpsimd
                    eng.dma_start(out=out[s0:s0 + 128, :], in_=o_sb)

            # prologue: minimal projections for (0,0) sk0; DMA order is
            # priority order on the serial DMA device
            load_x(0, 0)
            load_x(0, 1)
            load_x(1, 0)
            # warm the PE p-state with junk matmuls while the first x
            # chunks stream in: after ~3us of continuous execution the
            # tensor engine runs at 2.4GHz instead of 1.2
            junk = cst_pool.tile([1, 640], BF16, tag="junk")
            nc.vector.memset(junk, 1.0)
            for w in range(14):
                ps_w = psC.tile([128, SQW], F32, tag="psC", name=f"warm{w}")
                nc.tensor.matmul(ps_w, junk[:, 0:128], junk[:, 128:640],
                                 start=True, stop=True, skip_group_check=True)
            nc.vector.tensor_copy(
                bqk, pk16[:, KT * 128 + KM:KT * 128 + KM + 2 * MT])
            emit_qk_mm(0, 0, 0)
            flush_stg()
            load_x(2, 0)
            emit_qk_mm(0, 0, 1, pool=psM)
            flush_stg()
            load_x(1, 1)
            emit_qk_mm(1, 0, 0, pool=psS)
            flush_stg()
            load_x(2, 1)

            def emit_ctx(sk, e0, e1, ps_ch, ci, hp, cw):
                if not ps_ch:
                    for h in range(cw // SQW):
                        ps_ch.append(psC.tile([128, SQW], F32, tag="psC",
                                              name=f"psc{ci}_{hp}_{h}"))
                for h4 in range(cw // SQW):
                    qs = slice(h4 * SQW, (h4 + 1) * SQW)
                    nc.tensor.matmul(ps_ch[h4][0:64, :], vt[sk][:, hp, 0:HEAD],
                                     e0[:, qs], start=(sk == 0),
                                     stop=(sk == ST - 1), skip_group_check=True)
                    nc.tensor.matmul(ps_ch[h4][64:128, :], vt[sk][:, hp, HEAD:],
                                     e1[:, qs], start=(sk == 0),
                                     stop=(sk == ST - 1), skip_group_check=True)

            pending = None          # finished chunk awaiting normalize
            pending_out = None      # out-proj rounds due (generator)
            chunks = [(0, 0, 0, CW), (0, 1, 0, CW), (0, 2, 0, CW),
                      (1, 0, CW, CW), (1, 1, CW, CW), (1, 2, CW, CW)]
            for ci, (cid, hp, q0, cw) in enumerate(chunks):
                nh4 = cw // SQW
                ps_ch = []
                acc0 = [acc_pool.tile([128, CW], BF16, tag=f"acc0_{i}",
                                      name=f"acc0_{cid}{hp}{i}")
                        for i in range(NACC)]
                acc1 = [acc_pool.tile([128, CW], BF16, tag=f"acc1_{i}",
                                      name=f"acc1_{cid}{hp}{i}")
                        for i in range(NACC)]

                prev = None
                for sk in range(ST):
                    sks = slice(sk * 128, (sk + 1) * 128)
                    ps_s0 = psS.tile([128, CW], F32, tag="psS")
                    ps_s1 = psS.tile([128, CW], F32, tag="psS")
                    e0 = e0_pool.tile([128, CW], BF16, tag="e0")
                    e1 = e1_pool.tile([128, CW], BF16, tag="e1")
                    hs = slice(32 * hp, 32 * hp + 32)
                    for h4 in range(nh4):
                        sq = slice(q0 + h4 * SQW, q0 + (h4 + 1) * SQW)
                        qs = slice(h4 * SQW, (h4 + 1) * SQW)
                        nc.tensor.matmul(ps_s0[:, qs], kt8[hs, 0, :, sks],
                                         qt8[hs, 0, :, sq], perf_mode=DR)
                        nc.tensor.matmul(ps_s1[:, qs], kt8[hs, 1, :, sks],
                                         qt8[hs, 1, :, sq], perf_mode=DR)
                    nc.scalar.activation(e0[:, 0:cw], ps_s0[:, 0:cw], EXP,
                                         scale=0.125)
                    nc.scalar.activation(e1[:, 0:cw], ps_s1[:, 0:cw], EXP,
                                         scale=0.125)
                    flush_stg()
                    for fn in slot.get((cid, hp, sk), ()):
                        fn()
                    if prev is not None:
                        emit_ctx(prev[0], prev[1], prev[2], ps_ch, ci, hp, cw)
                    # denominator partials on DVE (bf16 = 2x mode);
                    # chain i covers sk 8i..8i+7
                    ci8 = sk // 8
                    if sk % 8 == 1:
                        nc.vector.tensor_tensor(acc0[ci8][:, 0:cw],
                                                prev[1][:, 0:cw],
                                                e0[:, 0:cw], op=ADD)
                        nc.vector.tensor_tensor(acc1[ci8][:, 0:cw],
                                                prev[2][:, 0:cw],
                                                e1[:, 0:cw], op=ADD)
                    elif sk % 8 >= 2:
                        nc.vector.tensor_tensor(acc0[ci8][:, 0:cw],
                                                acc0[ci8][:, 0:cw],
                                                e0[:, 0:cw], op=ADD)
                        eng = nc.gpsimd if (sk % 8 in (3, 5)
                                            and ci < 5) else nc.vector
                        eng.tensor_tensor(acc1[ci8][:, 0:cw],
                                          acc1[ci8][:, 0:cw],
                                          e1[:, 0:cw], op=ADD)
                    prev = (sk, e0, e1)
                    if sk == 3 and pending is not None:
                        pending_rr = emit_norm_reduce(pending)
                    if sk == 5 and pending is not None:
                        emit_norm_bcast(pending, pending_rr, 0)
                    if sk == 7 and pending is not None:
                        if pending[5] > 1:
                            emit_norm_bcast(pending, pending_rr, 1)
                        pending = None
                    if sk >= 8 and pending_out is not None:
                        if next(pending_out, StopIteration) is StopIteration:
                            pending_out = None
                emit_ctx(prev[0], prev[1], prev[2], ps_ch, ci, hp, cw)
                pending = (q0, hp, ps_ch, acc0, acc1, nh4)
                if ci == 2:
                    pending_out = outproj_rounds(0, 8, at_tail=False)
            # tail: normalize the last chunk, then the sc1 out-proj rounds
            rr_last = emit_norm_reduce(pending)
            emit_norm_bcast(pending, rr_last, 0)
            emit_norm_bcast(pending, rr_last, 1)
            for _ in outproj_rounds(CW, 8, at_tail=True):
                pass


def _cst_host():
    # [2,128] broadcast selector at cols 226:354: row1 -> even-head rows
    # 0:64, row0 -> odd-head rows 64:128 (r holds [recip_o; recip_e])
    c = np.zeros((128, 354), np.float32)
    c[:, 0] = 1.0      # M=1 ones reduce column
    c[:, 33] = 1.0     # row 32 of the zero-padded M=33 reduce
    c[1, 226:290] = 1.0
    c[0, 290:354] = 1.0
    return c


def kernel(query, key, value, wq, bq, wk, bk, wv, bv, wo, bo):
    global _NC, LAST_RESULTS, _LAST_IN_MAPS
    if _NC is None:
        _NC = _build()

    BF = ml_dtypes.bfloat16

    def bfc(a):
        return np.ascontiguousarray(np.asarray(a, dtype=np.float32).astype(BF))

    query, key, value = map(np.asarray, (query, key, value))
    xt = [{"xqt": bfc(query[b].T), "xkt": bfc(key[b].T),
           "xvt": bfc(value[b].T)} for b in range(B)]

    def mmaj(w):
        # [D, MC] -> [128, KT*MC] with per-partition [m, k, j] layout
        return w.reshape(KT, 128, MT, 128).transpose(1, 2, 0, 3).reshape(
            128, KT * MC)

    cst_h = _cst_host()
    wslices = []
    for g in range(2):
        cols = slice(g * MC, (g + 1) * MC)
        wq_g = np.asarray(wq, np.float32)[:, cols]
        wk_g = np.asarray(wk, np.float32)[:, cols]
        wv_g = np.asarray(wv, np.float32)[:, cols]
        wo_g = np.asarray(wo, np.float32)[cols, :]
        bq_g = np.asarray(bq, np.float32)[cols]
        bk_g = np.asarray(bk, np.float32)[cols]
        wq_m, wk_m = mmaj(wq_g), mmaj(wk_g)
        cstb_h = np.zeros((128, 68), np.float32)
        cstb_h[:, 1:34] = cst_h[:, 1:34]     # odd reduce: ones at col 33
        cstb_h[:, 67] = 1.0                  # even reduce: ones -> row 33
        # pack A: [wq-m0 | wk-m0 | wv(all) | bq | bk | cstb];
        # pack B: [wq-m1, wq-m2 | wk-m1, wk-m2]
        pk16 = np.concatenate(
            [wq_m[:, 0:KT * 128], wk_m[:, 0:KT * 128], mmaj(wv_g),
             bq_g.reshape(MT, 128).T, bk_g.reshape(MT, 128).T,
             cstb_h,
             wq_m[:, KT * 128:], wk_m[:, KT * 128:]],
            axis=1).astype(BF)
        pk32 = np.concatenate(
            [wo_g.reshape(MT, 128, D).transpose(1, 0, 2).reshape(128, MT * D),
             cst_h],
            axis=1).astype(np.float32)
        wslices.append({"wpk16": np.ascontiguousarray(pk16),
                        "wpk32": np.ascontiguousarray(pk32)})
    in_maps = [dict(xt[c // 2], **wslices[c % 2]) for c in range(NCORES)]

    _LAST_IN_MAPS = in_maps
    res = run_bass_kernel_spmd(_NC, in_maps, core_ids=list(range(NCORES)))
    LAST_RESULTS = res

    # host epilogue: pairwise partial-sum reduce + biases (bv@wo is exact
    # because softmax rows sum to 1, so ctx absorbs bv additively)
    corr = (np.asarray(bv, np.float64) @ np.asarray(wo, np.float64)
            + np.asarray(bo, np.float64)).astype(np.float32)
    y = np.empty((B, S, D), np.float32)
    for b in range(B):
        y[b] = res.results[2 * b]["out"] + res.results[2 * b + 1]["out"] + corr
    return y



# revision 46
# speedup vs baseline: 1.4307x; 1.4307x over previous
"""Multi-head attention (B=4, S=2048, D=768, H=12, d=64) on 8 trn2 NeuronCores.

Sharding: core c handles batch b = c//2 and head-group g = c%2 (6 heads each).
Per core: column-parallel QKV projections (wq/wk/wv column slices), full
attention for its 6 heads, row-parallel output projection (wo row slice).
The two partial outputs per batch are reduced on the host (+ bo and the
bv @ wo correction, exact because softmax rows sum to 1).

The kernel is ACT-bound: exp of 25.2M scores costs ~199us on the scalar
engine and nothing else can compute exp. Everything is arranged to hide
under that: scores run as fp8e4m3 DoubleRow matmuls (half PE cost, the
e4m3 q/k rounding costs ~1.36e-2 rel err), projections stream in bf16
and are interleaved quantum-by-quantum into the attention sk-loop via a
static slot schedule so the PE never bursts long enough to starve the
ACT engine. e/V tiles are bf16 (DVE 2x adds for the softmax partials,
split into short chains). Key structure:
 - ramp: a small leading weight pack (wq/wk m0 + biases) then the four
   first-exp-critical x chunks own the serial DMA wire; wv/xv/late packs
   are held back by WAW junk-write gates so they cannot steal wire slots
   (dep-free DMAs otherwise get scheduled ahead of the waiting shuffle
   DMAs). First exp fires at ~18us instead of ~25us.
 - psum chains: V projection quanta run on their own bank (psV) so the
   V chain and the W-projection/norm chain (psM) advance independently;
   ctx accumulators (psC) shrink to 2 banks, with the previous chunk's
   softmax-normalize moved to sks 1-3 and ctx delayed 4 sks (the last 4
   ctx emissions of each chunk carry into the next chunk's sk0/1, after
   its scores, so chunk boundaries never stall the exp stream).
 - tail: the last chunk's denominator reduce is split (sk0-7 partials
   fold into psum at sk9; sk15's e tiles feed the reduce directly so
   nothing waits on the final DVE add), the two norm halves run in
   parallel psum banks (psM/psV), and the output is stored fp16, which
   halves the wire-bound out-DMA time. ctx / denominator-reduce /
   output projection stay float32r.
TimelineSim: 257.6us (session start 270.9us, original stub 308.6us).
"""
import sys

for _p in ("/opt/trn_rl_repo", "/root/.axon_site/_ro/trn_rl_repo"):
    if _p not in sys.path:
        sys.path.append(_p)

import numpy as np
import ml_dtypes

import concourse.bass as bass  # noqa: F401  (engine namespaces live on the nc object)
import concourse.bacc as bacc
import concourse.mybir as mybir
import concourse.tile as tile
from concourse.bass_utils import run_bass_kernel_spmd

B, S, D = 4, 2048, 768
NUM_HEADS, HEAD = 12, 64
NCORES = 8
HPC = NUM_HEADS // 2          # 6 heads per core
MC = HPC * HEAD               # 384 per-core projection cols
KT = D // 128                 # 6 contraction k-tiles
MT = MC // 128                # 3 head-pair tiles
ST = S // 128                 # 16 key tiles
SQW = 512                     # psum bank width in f32
CW = 1024                     # attention q-chunk width
NCH = S // CW                 # 2 q-chunks
CHW = 512                     # x streaming chunk width
NXC = S // CHW                # 4 x chunks
NACC = 2                      # denominator partial chains per head

F32 = mybir.dt.float32
F32R = mybir.dt.float32r
F16 = mybir.dt.float16
BF16 = mybir.dt.bfloat16
F8 = mybir.dt.float8e4
DR = mybir.MatmulPerfMode.DoubleRow
EXP = mybir.ActivationFunctionType.Exp
ADD = mybir.AluOpType.add
MULT = mybir.AluOpType.mult

_NC = None
LAST_RESULTS = None
_LAST_IN_MAPS = None


def _build(loop=None):
    nc = bacc.Bacc("TRN2", target_bir_lowering=False, debug=False,
                   num_devices=NCORES)
    xqt = nc.declare_dram_parameter("xqt", [D, S], BF16, isOutput=False)
    xkt = nc.declare_dram_parameter("xkt", [D, S], BF16, isOutput=False)
    xvt = nc.declare_dram_parameter("xvt", [D, S], BF16, isOutput=False)
    # bf16 pack: [wq_m0 | wk_m0 | bq | bk | cstb(68)] leads (everything the
    # first q/k projections + quantize need in one small DMA), then [wv],
    # then [wq_m1,m2 | wk_m1,m2]; f32 pack: [wo | cst]
    wpk16 = nc.declare_dram_parameter("wpk16", [128, 3 * KT * MC + 74], BF16,
                                      isOutput=False)
    wpk32 = nc.declare_dram_parameter("wpk32", [128, MT * D + 354],
                                      F32R, isOutput=False)
    out = nc.declare_dram_parameter("out", [S, D], F16, isOutput=True)

    with tile.TileContext(nc) as tc:
        if loop:
            with tc.For_i(0, loop, 1):
                _emit(nc, tc, xqt, xkt, xvt, wpk16, wpk32, out)
        else:
            _emit(nc, tc, xqt, xkt, xvt, wpk16, wpk32, out)
    nc.compile()
    return nc


def _emit(nc, tc, xqt, xkt, xvt, wpk16, wpk32, out):
    ctx_lp = nc.allow_low_precision(reason="fp8/bf16 tiles feed the PE; accumulation stays fp32 in PSUM")
    ctx_lp.__enter__()
    with (
        tc.tile_pool(name="cst", bufs=1) as cst_pool,
        tc.tile_pool(name="q8p", bufs=1) as q8_pool,
        tc.tile_pool(name="vp", bufs=ST) as v_pool,
        tc.tile_pool(name="ctxp", bufs=MT) as ctx_pool,
        tc.tile_pool(name="accp", bufs=2) as acc_pool,
        tc.tile_pool(name="rp", bufs=2) as r_pool,
        tc.tile_pool(name="wop", bufs=1) as wo_pool,
        tc.tile_pool(name="outp", bufs=4) as out_pool,
        tc.tile_pool(name="wp", bufs=1) as w_pool,
        tc.tile_pool(name="xtp", bufs=10) as xt_pool,
        tc.tile_pool(name="stgp", bufs=4) as stg_pool,
        tc.tile_pool(name="psS", bufs=2, space="PSUM") as psS,
        tc.tile_pool(name="psC", bufs=2, space="PSUM") as psC,
        tc.tile_pool(name="psV", bufs=1, space="PSUM") as psV,
        tc.tile_pool(name="psM", bufs=1, space="PSUM") as psM,
    ):
        psA = psM
        # packed weight/constant tiles; views are carved out below.
        # cst layout: [:,0:1]=ones  [:,1:34]=[32 zero cols|ones]
        # [:,34:98]=ones  [:,98:226]=[64 zero cols|64 one cols]
        pk16h = cst_pool.tile([128, 2 * KT * 128 + 74], BF16, tag="pk16h")
        pk16 = cst_pool.tile([128, KT * MC], BF16, tag="pk16")
        pk16b = cst_pool.tile([128, 4 * KT * 128], BF16, tag="pk16b")
        pk32 = cst_pool.tile([128, MT * D + 354], F32R, tag="pk32")

        # fp8 DoubleRow q/k: [32*hp+p, head, i, s] = proj_col(64*head+32*i+p
        # of pair hp) at seq s; scores lhsT/rhs slice partitions 32*hp..+32.
        qt8 = q8_pool.tile([96, 2, 2, S], F8, tag="qt8")
        kt8 = q8_pool.tile([96, 2, 2, S], F8, tag="kt8")
        # V tiles: [key_in_block, hp, 2*HEAD] (even head | odd head)
        vt = [v_pool.tile([128, MT, 2 * HEAD], BF16, tag="v", name=f"vt{st}")
              for st in range(ST)]
        ctx = [ctx_pool.tile([128, S], F32R, tag="ctx", name=f"ctx{m}")
               for m in range(MT)]

        # pack A (pk16h) = [wq-m0 | wk-m0 | bq | bk | cstb]: everything the
        # first q/k projections + quantize need, in one small leading DMA on
        # the SP/HWDGE queue (serial in the cost model, so order = priority).
        # pack wv (pk16) + pack B (pk16b = [wq-m1,m2 | wk-m1,m2]) + pk32 ride
        # the gpsimd/SWDGE path, which bypasses the serial DMA device.
        KM = KT * MC
        HPRE = 2 * KT * 128 + 74
        PRE = HPRE + KM
        nc.sync.dma_start(out=pk16h, in_=wpk16[:, 0:HPRE])

        def w_ap(name, k, m):
            # per-partition layout within each weight block: [m, k, j]
            if m == 0:
                o = (0 if name == "wq" else KT * 128) + k * 128
                return pk16h[:, o:o + 128]
            o = (0 if name == "wq" else 2 * KT * 128) + \
                ((m - 1) * KT + k) * 128
            return pk16b[:, o:o + 128]

        bqk = cst_pool.tile([128, 2 * MT], F32, tag="bqk")
        bq_sb = bqk[:, 0:MT]
        bk_sb = bqk[:, MT:2 * MT]
        # cstb: [1:34] odd-head 33-col reduce (ones at last col -> row 32),
        # [34:67] even-head (ones at col 31 -> row 31)
        cstb = pk16h[:, 2 * KT * 128 + 6:2 * KT * 128 + 74]
        wv4 = pk16[:, 0:KM].rearrange(
            "p (m k j) -> p k m j", m=MT, k=KT)
        wo_sb = pk32[:, 0:MT * D].rearrange("p (t o) -> p t o", t=MT)
        cst = pk32[:, MT * D:MT * D + 354]

        # ---------------- streaming projections ----------------
        x_dram = {0: xqt, 1: xkt, 2: xvt}
        x_tiles = {}

        def load_x(inp, c, eng=None, gate=None):
            x = xt_pool.tile([128, KT, CHW], BF16, tag="xt",
                             name=f"x{inp}_{c}")
            if gate is not None:
                # junk write into the destination creates a WAW hazard that
                # holds this DMA behind `gate` (overwritten by the DMA)
                nc.gpsimd.tensor_copy(x[0:1, 0, 0:8], gate)
            (eng or nc.sync).dma_start(
                out=x,
                in_=x_dram[inp][:].rearrange("(t p) s -> p t s", p=128)[
                    :, :, c * CHW:(c + 1) * CHW])
            x_tiles[(inp, c)] = x

        pending_stg = []

        def emit_qk_mm(inp, m, c, pool=None):
            # quantum phase 1: the projection matmul chain
            x = x_tiles[(inp, c)]
            wname = "wq" if inp == 0 else "wk"
            pool = pool or psA
            ps = pool.tile([128, SQW] if pool is not psS else [128, CW],
                           F32, tag="psS" if pool is psS else "psM",
                           name=f"psp{inp}{m}{c}")
            ps = ps[:, 0:SQW]
            for k in range(KT):
                nc.tensor.matmul(ps, w_ap(wname, k, m),
                                 x[:, k, :], start=(k == 0), stop=(k == KT - 1))
            pending_stg.append((inp, m, c, ps))

        def flush_stg():
            # quantum phase 2: fp8 quantize + shuffle-DMAs; runs one slot
            # later so the DVE op sits at the queue head, not behind adds
            while pending_stg:
                inp, m, c, ps = pending_stg.pop(0)
                bias_sb = bq_sb if inp == 0 else bk_sb
                dst = qt8 if inp == 0 else kt8
                stg = stg_pool.tile([128, CHW], F8, tag="stg")
                nc.vector.tensor_scalar_add(stg, ps, bias_sb[:, m:m + 1])
                for head in range(2):
                    for i in range(2):
                        nc.sync.dma_start(
                            out=dst[32 * m:32 * m + 32, head, i,
                                    c * CHW:(c + 1) * CHW],
                            in_=stg[64 * head + 32 * i:
                                    64 * head + 32 * i + 32, :])

        def emit_qk(inp, m, c):
            emit_qk_mm(inp, m, c)
            flush_stg()

        def emit_v(c, st8):
            # one key-block of the V projection; psv's (m, head, d) flat
            # layout matches vt exactly, so a single copy moves it all.
            # V rides its own psum bank (psV) so the V chain and the W/norm
            # chain (psM) advance independently.
            x = x_tiles[(2, c)]
            st = c * (CHW // 128) + st8
            ps = psV.tile([128, SQW], F32, tag="psV", name=f"psv{st}")
            for k in range(KT):
                nc.tensor.matmul(ps[:, 0:MC],
                                 x[:, k, st8 * 128:(st8 + 1) * 128],
                                 wv4[:, k],
                                 start=(k == 0), stop=(k == KT - 1))
            nc.vector.tensor_copy(
                vt[st], ps[:, 0:MC].rearrange("p (t x) -> p t x", x=2 * HEAD))

        # Static slot schedule: work items emitted at the top of sk
        # iterations so projection quanta fill the PE slack under the
        # ACT-paced exp stream without ever bursting. Constraints held:
        # x-chunk 8-buf rotation distance, kt8/vt ready >=2 slots before
        # first use, qt8[hp] ready before chunk (sc,hp) starts.
        slot = {}

        def at(sc, hp, sk, *fns):
            slot.setdefault((sc, hp, sk), []).extend(fns)

        W = lambda inp, m, c: (lambda: emit_qk(inp, m, c))
        V = lambda c, s8: (lambda: emit_v(c, s8))
        L = lambda inp, c: (lambda: load_x(inp, c))
        # (0,0): the psM chain (V/W quanta, one bank) flows in slot order;
        # W(1,0,c) leads its slot so kt8 stays ahead of the sk loop, V
        # singles trail one per slot just-in-time for ctx. Non-urgent W
        # quanta (q/k m1/m2 tails) are pushed into chunks (0,1)/(0,2).
        # Pending-norm chunks keep quanta off sk 3/5/7 (psM conflicts);
        # (0,0) has no pending norm so those slots are usable here.
        at(0, 0, 0, W(1, 0, 1), V(0, 0))
        at(0, 0, 1, V(0, 1))
        at(0, 0, 2, L(1, 2), V(0, 2))
        at(0, 0, 3, L(2, 2), V(0, 3))
        at(0, 0, 4, W(1, 0, 2), V(1, 0))
        at(0, 0, 5, L(1, 3), V(1, 1))
        at(0, 0, 6, W(0, 1, 0), V(1, 2))
        at(0, 0, 7, V(1, 3))
        at(0, 0, 8, W(1, 0, 3), V(2, 0))
        at(0, 0, 9, L(2, 3), V(2, 1))
        at(0, 0, 10, W(0, 1, 1), V(2, 2))
        at(0, 0, 11, V(2, 3))
        at(0, 0, 12, W(1, 1, 0), V(3, 0))
        at(0, 0, 13, V(3, 1))
        at(0, 0, 14, V(3, 2), V(3, 3))
        # (0,1): k m1 tail, q/k m2, q c2/c3 for sc1. Pending-norm chunks
        # keep W quanta off sks 1-3 (norm owns psM there).
        at(0, 1, 0, W(1, 1, 1))
        at(0, 1, 2, L(0, 2))
        at(0, 1, 4, L(0, 3), W(1, 1, 2))
        at(0, 1, 5, W(1, 1, 3))
        at(0, 1, 6, W(0, 2, 0))
        at(0, 1, 7, W(0, 2, 1))
        at(0, 1, 8, W(1, 2, 0))
        at(0, 1, 9, W(1, 2, 1))
        at(0, 1, 10, W(1, 2, 2))
        at(0, 1, 11, W(1, 2, 3))
        at(0, 1, 12, W(0, 0, 2))
        at(0, 1, 13, W(0, 0, 3))
        at(0, 1, 14, W(0, 1, 2))
        at(0, 2, 0, W(0, 1, 3))
        at(0, 2, 4, W(0, 2, 2))
        at(0, 2, 5, W(0, 2, 3))

        # ---------------- attention ----------------
        with (
            tc.tile_pool(name="ep0", bufs=8) as e0_pool,
            tc.tile_pool(name="ep1", bufs=8) as e1_pool,
        ):
            def emit_norm_reduce(state):
                # stage 1: partition-reduce matmuls + reciprocals
                q0, hp, ps_ch, acc0, acc1, nh4 = state
                rr = []
                for h4 in range(nh4):
                    qs = slice(h4 * SQW, (h4 + 1) * SQW)
                    ps_r = psM.tile([34, SQW], F32, tag="psM",
                                    name=f"psr{q0}{hp}{h4}")
                    for i in range(NACC):
                        nc.tensor.matmul(ps_r, cstb[:, 34:68],
                                         acc0[i][:, qs],
                                         start=(i == 0), stop=False,
                                         skip_group_check=True)
                        nc.tensor.matmul(ps_r[0:33, :], cstb[:, 1:34],
                                         acc1[i][:, qs],
                                         start=False, stop=(i == NACC - 1),
                                         skip_group_check=True)
                    r = r_pool.tile([2, SQW], F32R, tag="r")
                    nc.vector.reciprocal(r, ps_r[32:34, :])
                    rr.append(r)
                return rr

            def tail_pool(h4):
                # at the tail the V chain is done, so psV gives the odd half
                # its own bank and the two norm halves run concurrently
                return (psM, "psM") if h4 == 0 else (psV, "psV")

            def norm_reduce_pre(state):
                # last chunk: the sk0-7 partials are final at sk9 -- fold
                # them into psum early so only the i=1 pass sits in the tail
                q0, hp, ps_ch, acc0, acc1, nh4 = state
                tiles = []
                for h4 in range(nh4):
                    qs = slice(h4 * SQW, (h4 + 1) * SQW)
                    pool, tag = tail_pool(h4)
                    ps_r = pool.tile([34, SQW], F32, tag=tag,
                                     name=f"psrT{h4}")
                    nc.tensor.matmul(ps_r, cstb[:, 34:68], acc0[0][:, qs],
                                     start=True, stop=False,
                                     skip_group_check=True)
                    nc.tensor.matmul(ps_r[0:33, :], cstb[:, 1:34],
                                     acc1[0][:, qs],
                                     start=False, stop=False,
                                     skip_group_check=True)
                    tiles.append(ps_r)
                return tiles

            def norm_reduce_fin(state, tiles, edir=None):
                # edir: sk15's raw e tiles folded straight into the psum
                # reduce, so the tail never waits on the final DVE acc add
                q0, hp, ps_ch, acc0, acc1, nh4 = state
                rr = []
                for h4 in range(nh4):
                    qs = slice(h4 * SQW, (h4 + 1) * SQW)
                    ps_r = tiles[h4]
                    srcs = [(acc0[1], acc1[1])]
                    if edir is not None:
                        srcs.append(edir)
                    for j, (a0, a1) in enumerate(srcs):
                        last = j == len(srcs) - 1
                        nc.tensor.matmul(ps_r, cstb[:, 34:68], a0[:, qs],
                                         start=False, stop=False,
                                         skip_group_check=True)
                        nc.tensor.matmul(ps_r[0:33, :], cstb[:, 1:34],
                                         a1[:, qs],
                                         start=False, stop=last,
                                         skip_group_check=True)
                    r = r_pool.tile([2, SQW], F32R, tag="r")
                    nc.vector.reciprocal(r, ps_r[32:34, :])
                    rr.append(r)
                return rr

            def emit_norm_bcast(state, rr, h4, pool=None):
                # stage 2: broadcast matmul + normalize into ctx (one half)
                q0, hp, ps_ch, acc0, acc1, nh4 = state
                r2 = rr[h4]
                pl, tag = (pool, "psV") if pool is not None else (psM, "psM")
                ps_b = pl.tile([128, SQW], F32, tag=tag,
                               name=f"psb{q0}{hp}{h4}")
                nc.tensor.matmul(ps_b, cst[0:2, 226:354], r2,
                                 start=True, stop=True,
                                 skip_group_check=True)
                b_sb = r_pool.tile([128, SQW], F32, tag="bsb", bufs=2)
                nc.vector.tensor_copy(b_sb, ps_b)
                s0 = q0 + h4 * SQW
                nc.vector.tensor_tensor(ctx[hp][:, s0:s0 + SQW],
                                        ps_ch[h4], b_sb, op=MULT)

            def outproj_rounds(row0, nr, at_tail):
                # one round = one 128-row out block; two yields per round
                # mid-stream, interleaved into sk slots. At the tail, rounds
                # use the freed scores banks (psS) + psM and split their
                # psum->sbuf copies between ACT (idle then) and DVE.
                for st4 in range(nr):
                    s0 = row0 + st4 * 128
                    o_sb = out_pool.tile([128, D], F16, tag="osb")
                    if at_tail and st4 % 4 in (1, 2):
                        ps_o = psS.tile([128, CW], F32, tag="psS",
                                        name=f"psot{s0}")[:, 0:768]
                        for n0, nw in ((0, 512), (512, 256)):
                            for m in range(MT):
                                nc.tensor.matmul(
                                    ps_o[:, n0:n0 + nw],
                                    ctx[m][:, s0:s0 + 128],
                                    wo_sb[:, m, n0:n0 + nw],
                                    start=(m == 0), stop=(m == MT - 1))
                            yield
                        if st4 % 2:
                            nc.scalar.copy(o_sb, ps_o)
                        else:
                            nc.vector.tensor_copy(o_sb, ps_o)
                    else:
                        for n0, nw in ((0, 512), (512, 256)):
                            pool = psC if (at_tail and st4 % 4 == 3) else psM
                            ps_o = pool.tile([128, 512], F32,
                                            tag="psC" if pool is psC else "psM",
                                            name=f"pso{s0}{n0}")
                            for m in range(MT):
                                nc.tensor.matmul(
                                    ps_o[:, 0:nw],
                                    ctx[m][:, s0:s0 + 128],
                                    wo_sb[:, m, n0:n0 + nw],
                                    start=(m == 0), stop=(m == MT - 1))
                            if at_tail and st4 % 2:
                                nc.scalar.copy(o_sb[:, n0:n0 + nw],
                                               ps_o[:, 0:nw])
                            else:
                                nc.vector.tensor_copy(o_sb[:, n0:n0 + nw],
                                                      ps_o[:, 0:nw])
                            yield
                    eng = nc.sync if at_tail else nc.gpsimd
                    eng.dma_start(out=out[s0:s0 + 128, :], in_=o_sb)

            # prologue: minimal projections for (0,0) sk0. The DMA wire is a
            # serial device, so SP emission order = wire priority: the
            # first-exp critical chain (xq c0/c1, xk c0) plus the three
            # quantize-shuffle batches go first; xv c0 / wv / xk c1 / xv c1
            # queue behind them. Mid-stream loads issue via gpsimd so the SP
            # queue stays free for shuffles.
            load_x(0, 0, eng=nc.sync)
            load_x(1, 0, eng=nc.sync)
            load_x(0, 1, eng=nc.sync)
            load_x(1, 1, eng=nc.sync)
            # warm the PE p-state with junk matmuls while the first x
            # chunks stream in: after ~3us of continuous execution the
            # tensor engine runs at 2.4GHz instead of 1.2
            junk = cst_pool.tile([1, 640], BF16, tag="junk")
            nc.vector.memset(junk, 1.0)
            for w in range(6):
                ps_w = psV.tile([128, SQW], F32, tag="psV", name=f"warm{w}")
                nc.tensor.matmul(ps_w, junk[:, 0:128], junk[:, 128:640],
                                 start=True, stop=True, skip_group_check=True)
            nc.vector.tensor_copy(
                bqk, pk16h[:, 2 * KT * 128:2 * KT * 128 + 2 * MT])
            emit_qk_mm(0, 0, 0)
            flush_stg()
            emit_qk_mm(1, 0, 0, pool=psS)
            flush_stg()
            emit_qk_mm(0, 0, 1, pool=psM)
            flush_stg()
            # wv/xv/pk16b/pk32 are held behind the first-exp chain by WAW
            # gates: a junk write (sourced from qt8, ready only after the
            # c0 shuffles) into each destination tile forces the DMA to
            # wait, so it cannot steal wire slots from the critical loads.
            gsrc = qt8[0:1, 0, 0, 0:8]
            nc.gpsimd.tensor_copy(pk16[0:1, 0:8], gsrc)
            nc.gpsimd.dma_start(out=pk16, in_=wpk16[:, HPRE:PRE])
            load_x(2, 0, eng=nc.gpsimd, gate=gsrc)
            load_x(2, 1, eng=nc.gpsimd, gate=gsrc)
            nc.gpsimd.tensor_copy(pk16b[0:1, 0:8], gsrc)
            nc.sync.dma_start(out=pk16b, in_=wpk16[:, PRE:])
            nc.gpsimd.tensor_copy(pk32[0:1, 0:4], qt8[0:1, 0, 0, 0:4])
            nc.sync.dma_start(out=pk32, in_=wpk32[:])

            def emit_ctx(sk, e0, e1, ps_ch, ci, hp, cw):
                if not ps_ch:
                    for h in range(cw // SQW):
                        ps_ch.append(psC.tile([128, SQW], F32, tag="psC",
                                              name=f"psc{ci}_{hp}_{h}"))
                for h4 in range(cw // SQW):
                    qs = slice(h4 * SQW, (h4 + 1) * SQW)
                    nc.tensor.matmul(ps_ch[h4][0:64, :], vt[sk][:, hp, 0:HEAD],
                                     e0[:, qs], start=(sk == 0),
                                     stop=(sk == ST - 1), skip_group_check=True)
                    nc.tensor.matmul(ps_ch[h4][64:128, :], vt[sk][:, hp, HEAD:],
                                     e1[:, qs], start=(sk == 0),
                                     stop=(sk == ST - 1), skip_group_check=True)

            pending = None          # finished chunk awaiting normalize
            pending_out = None      # out-proj rounds due (generator)
            carried = []            # prev chunk's last ctx emissions, popped
            chunks = [(0, 0, 0, CW), (0, 1, 0, CW), (0, 2, 0, CW),
                      (1, 0, CW, CW), (1, 1, CW, CW), (1, 2, CW, CW)]
            for ci, (cid, hp, q0, cw) in enumerate(chunks):
                nh4 = cw // SQW
                ps_ch = []
                acc0 = [acc_pool.tile([128, CW], BF16, tag=f"acc0_{i}",
                                      name=f"acc0_{cid}{hp}{i}")
                        for i in range(NACC)]
                acc1 = [acc_pool.tile([128, CW], BF16, tag=f"acc1_{i}",
                                      name=f"acc1_{cid}{hp}{i}")
                        for i in range(NACC)]

                prev = None
                pend = []      # e tiles awaiting ctx: delayed 4 sks so the
                cdel = 4       # previous chunk's norm (sk1-3) finishes with
                for sk in range(ST):  # ps_ch before ctx reuses its banks
                    sks = slice(sk * 128, (sk + 1) * 128)
                    ps_s0 = psS.tile([128, CW], F32, tag="psS")
                    ps_s1 = psS.tile([128, CW], F32, tag="psS")
                    e0 = e0_pool.tile([128, CW], BF16, tag="e0")
                    e1 = e1_pool.tile([128, CW], BF16, tag="e1")
                    hs = slice(32 * hp, 32 * hp + 32)
                    for h4 in range(nh4):
                        sq = slice(q0 + h4 * SQW, q0 + (h4 + 1) * SQW)
                        qs = slice(h4 * SQW, (h4 + 1) * SQW)
                        nc.tensor.matmul(ps_s0[:, qs], kt8[hs, 0, :, sks],
                                         qt8[hs, 0, :, sq], perf_mode=DR)
                        nc.tensor.matmul(ps_s1[:, qs], kt8[hs, 1, :, sks],
                                         qt8[hs, 1, :, sq], perf_mode=DR)
                    nc.scalar.activation(e0[:, 0:cw], ps_s0[:, 0:cw], EXP,
                                         scale=0.125)
                    nc.scalar.activation(e1[:, 0:cw], ps_s1[:, 0:cw], EXP,
                                         scale=0.125)
                    flush_stg()
                    for fn in slot.get((cid, hp, sk), ()):
                        fn()
                    # prev chunk's ctx leftovers land in our sk0/1 (after our
                    # scores, so they never delay the exp stream)
                    if sk < 2:
                        for _ in range(2):
                            if carried:
                                carried.pop(0)()
                    drain = ci == len(chunks) - 1 and sk >= 10
                    for _ in range(2 if drain else 1):
                        if pend and (len(pend) >= cdel or drain):
                            p = pend.pop(0)
                            emit_ctx(p[0], p[1], p[2], ps_ch, ci, hp, cw)
                    # denominator partials on DVE (bf16 = 2x mode);
                    # chain i covers sk 8i..8i+7
                    ci8 = sk // 8
                    if sk % 8 == 1:
                        nc.vector.tensor_tensor(acc0[ci8][:, 0:cw],
                                                prev[1][:, 0:cw],
                                                e0[:, 0:cw], op=ADD)
                        nc.vector.tensor_tensor(acc1[ci8][:, 0:cw],
                                                prev[2][:, 0:cw],
                                                e1[:, 0:cw], op=ADD)
                    elif sk % 8 >= 2 and not (ci == len(chunks) - 1
                                              and sk == ST - 1):
                        # last chunk skips the sk15 adds: those e tiles go
                        # straight into the tail reduce matmuls instead
                        nc.vector.tensor_tensor(acc0[ci8][:, 0:cw],
                                                acc0[ci8][:, 0:cw],
                                                e0[:, 0:cw], op=ADD)
                        eng = nc.gpsimd if (sk % 8 in (3, 5)
                                            and ci < 5) else nc.vector
                        eng.tensor_tensor(acc1[ci8][:, 0:cw],
                                          acc1[ci8][:, 0:cw],
                                          e1[:, 0:cw], op=ADD)
                    prev = (sk, e0, e1)
                    pend.append(prev)
                    if sk == 1 and pending is not None:
                        pending_rr = emit_norm_reduce(pending)
                    if sk == 2 and pending is not None:
                        emit_norm_bcast(pending, pending_rr, 0)
                    if sk == 3 and pending is not None:
                        if pending[5] > 1:
                            emit_norm_bcast(pending, pending_rr, 1)
                        pending = None
                    if ci == len(chunks) - 1 and sk == 9:
                        tail_red = norm_reduce_pre((q0, hp, ps_ch,
                                                    acc0, acc1, nh4))
                    if sk >= 8 and pending_out is not None:
                        if next(pending_out, StopIteration) is StopIteration:
                            pending_out = None
                if ci < len(chunks) - 1:
                    def mk(p, ps_ch=ps_ch, ci=ci, hp=hp, cw=cw):
                        return lambda: emit_ctx(p[0], p[1], p[2],
                                                ps_ch, ci, hp, cw)
                    carried = [mk(p) for p in pend]
                else:
                    for p in pend:
                        emit_ctx(p[0], p[1], p[2], ps_ch, ci, hp, cw)
                pending = (q0, hp, ps_ch, acc0, acc1, nh4)
                if ci == 2:
                    pending_out = outproj_rounds(0, 8, at_tail=False)
            # tail: finish the last chunk's norm (halves in parallel psum
            # banks), then the sc1 out-proj rounds
            rr_last = norm_reduce_fin(pending, tail_red,
                                      edir=(prev[1], prev[2]))
            emit_norm_bcast(pending, rr_last, 0)
            emit_norm_bcast(pending, rr_last, 1, pool=psV)
            for _ in outproj_rounds(CW, 8, at_tail=True):
                pass


def _cst_host():
    # [2,128] broadcast selector at cols 226:354: row1 -> even-head rows
    # 0:64, row0 -> odd-head rows 64:128 (r holds [recip_o; recip_e])
    c = np.zeros((128, 354), np.float32)
    c[:, 0] = 1.0      # M=1 ones reduce column
    c[:, 33] = 1.0     # row 32 of the zero-padded M=33 reduce
    c[1, 226:290] = 1.0
    c[0, 290:354] = 1.0
    return c


def kernel(query, key, value, wq, bq, wk, bk, wv, bv, wo, bo):
    global _NC, LAST_RESULTS, _LAST_IN_MAPS
    if _NC is None:
        _NC = _build()

    BF = ml_dtypes.bfloat16

    def bfc(a):
        return np.ascontiguousarray(np.asarray(a, dtype=np.float32).astype(BF))

    query, key, value = map(np.asarray, (query, key, value))
    xt = [{"xqt": bfc(query[b].T), "xkt": bfc(key[b].T),
           "xvt": bfc(value[b].T)} for b in range(B)]

    def mmaj(w):
        # [D, MC] -> [128, KT*MC] with per-partition [m, k, j] layout
        return w.reshape(KT, 128, MT, 128).transpose(1, 2, 0, 3).reshape(
            128, KT * MC)

    cst_h = _cst_host()
    wslices = []
    for g in range(2):
        cols = slice(g * MC, (g + 1) * MC)
        wq_g = np.asarray(wq, np.float32)[:, cols]
        wk_g = np.asarray(wk, np.float32)[:, cols]
        wv_g = np.asarray(wv, np.float32)[:, cols]
        wo_g = np.asarray(wo, np.float32)[cols, :]
        bq_g = np.asarray(bq, np.float32)[cols]
        bk_g = np.asarray(bk, np.float32)[cols]
        wq_m, wk_m = mmaj(wq_g), mmaj(wk_g)
        cstb_h = np.zeros((128, 68), np.float32)
        cstb_h[:, 1:34] = cst_h[:, 1:34]     # odd reduce: ones at col 33
        cstb_h[:, 67] = 1.0                  # even reduce: ones -> row 33
        # pack A: [wq-m0 | wk-m0 | bq | bk | cstb]; then [wv(all)];
        # pack B: [wq-m1, wq-m2 | wk-m1, wk-m2]
        pk16 = np.concatenate(
            [wq_m[:, 0:KT * 128], wk_m[:, 0:KT * 128],
             bq_g.reshape(MT, 128).T, bk_g.reshape(MT, 128).T,
             cstb_h,
             mmaj(wv_g),
             wq_m[:, KT * 128:], wk_m[:, KT * 128:]],
            axis=1).astype(BF)
        pk32 = np.concatenate(
            [wo_g.reshape(MT, 128, D).transpose(1, 0, 2).reshape(128, MT * D),
             cst_h],
            axis=1).astype(np.float32)
        wslices.append({"wpk16": np.ascontiguousarray(pk16),
                        "wpk32": np.ascontiguousarray(pk32)})
    in_maps = [dict(xt[c // 2], **wslices[c % 2]) for c in range(NCORES)]

    _LAST_IN_MAPS = in_maps
    res = run_bass_kernel_spmd(_NC, in_maps, core_ids=list(range(NCORES)))
    LAST_RESULTS = res

    # host epilogue: pairwise partial-sum reduce + biases (bv@wo is exact
    # because softmax rows sum to 1, so ctx absorbs bv additively)
    corr = (np.asarray(bv, np.float64) @ np.asarray(wo, np.float64)
            + np.asarray(bo, np.float64)).astype(np.float32)
    y = np.empty((B, S, D), np.float32)
    for b in range(B):
        y[b] = (res.results[2 * b]["out"].astype(np.float32)
                + res.results[2 * b + 1]["out"].astype(np.float32) + corr)
    return y

